# revision 2
# baseline (speedup 1.0000x reference)
"""BertSelfAttention (B=4, S=2048, D=768, H=12) on 8 Trainium2 NeuronCores.

Sharding: 8 cores = 4 batches x 2 head-groups (6 heads each).

v3: all matmuls bf16 at K=128 contraction (measured ~138ns per 512-col
matmul on HW vs 570-716ns for f32r / K=64 tile modes), so ScalarE's exp
stream (~0.96us per [128,1024] op) is the bottleneck. Host casts to bf16.

Per core, for its (batch b, head-group g):

  QT[mi]  = (Wq_pair^T @ x_b^T)   [128, 2048] bf16, pair-stacked (2 heads)
  KH[h]   = zero-padded per-head K^T: rows (h%2)*64..+64 hold Kh^T, other
            rows zero, so the scores matmul contracts K=128 (fast tile
            mode) while summing only the real 64 dims.
  V[sc]   = x_b @ Wv (+ ones column per head for the softmax denominator)
  scores  S^T[k,q] = KH[h] chunk @ QT[mi]            (PE, K=128 bf16)
  P^T     = exp(S^T * 1/8 + mask[k])                 (ScalarE -> bf16)
  ctx,den = V_aug chunk @ P^T accumulated over kc    (PE, K=128 bf16)
  tail    : denom -> SBUF, 1/denom (DVE), broadcast across partitions
            (GPSIMD partition_broadcast, idle engine), ctx*recip (DVE)

Schedule:
 - input DMA split across both HWDGE rings (SP + Activation) to halve the
   DMA wall; V(0..3) + Q/K chains for head 0 run under the DMA, then the
   attention loop starts (~12us in) and never lets ScalarE idle.
 - the remaining chains (V4..15, rest of Q/K) are woven into the loop's
   PE slack. Because the PE queue is in-order, PV(bi,kc) emission is
   LAGGED ~8 iterations behind exp(bi,kc) (pt ring bufs=16 buffers the
   exp outputs), so a chain stalling on its DMA or a tail stalling on
   the ctx drain never sits in front of the scores the exp stream needs.
 - chains use a dedicated PSUM tag ("c" 2x[128,512]); ctx uses a single
   slot, freed right after the block tail's denominator-copy + drain.

Host side only slices/transposes/casts for layout - all FLOPs on device.
"""

import numpy as np
import ml_dtypes

import concourse.mybir as mybir
import concourse.tile as tile
from concourse import bacc
from concourse.bass_utils import run_bass_kernel_spmd

F32 = mybir.dt.float32
F32R = mybir.dt.float32r
BF16 = mybir.dt.bfloat16
U32 = mybir.dt.uint32
ONE_F32_BITS = 0x3F800000

N_CORES = 8
B, S, D, H = 4, 2048, 768, 12
HL = 6           # heads per core
DH = 64          # head dim
DL = HL * DH     # 384: local output dim
DCH = D // 128   # 6 contraction chunks
MCH = DL // 128  # 3 output-partition chunks (head pairs)
SCH = S // 128   # 16 seq chunks
VSTRIDE = 128    # per-head stride in the augmented-V tile
QW = 1024        # q-block width in the attention loop
QB = S // QW     # 2 q-blocks per head
NBLK = HL * QB   # 12 (head, q-block) attention blocks

_cached = {}


def build_program(reps=1):
    if ("nc", reps) in _cached:
        return _cached[("nc", reps)]
    nc = bacc.Bacc("TRN2", target_bir_lowering=False, debug=False, num_devices=1)
    xT = nc.dram_tensor("xT", [D, S], BF16, kind="ExternalInput").ap()
    wq = nc.dram_tensor("wq", [D, DL], BF16, kind="ExternalInput").ap()
    wk = nc.dram_tensor("wk", [D, DL], BF16, kind="ExternalInput").ap()
    wv = nc.dram_tensor("wv", [D, DL], BF16, kind="ExternalInput").ap()
    bq = nc.dram_tensor("bq", [128, MCH], F32, kind="ExternalInput").ap()
    bk = nc.dram_tensor("bk", [128, MCH], F32, kind="ExternalInput").ap()
    bv = nc.dram_tensor("bv", [1, DL], F32, kind="ExternalInput").ap()
    mask = nc.dram_tensor("mask", [128, SCH], F32, kind="ExternalInput").ap()
    out = nc.dram_tensor("out", [HL, DH, S], F32, kind="ExternalOutput").ap()

    EXP = mybir.ActivationFunctionType.Exp
    MULT = mybir.AluOpType.mult
    ADD = mybir.AluOpType.add

    blocks = [(h, qb) for h in range(HL) for qb in range(QB)]

    with tile.TileContext(nc) as tc:
      for _rep in range(reps):
        with tc.tile_pool(name="persist", bufs=1) as persist:
            QT = [persist.tile([128, S], BF16, tag=f"qt{i}", name=f"qt{i}")
                  for i in range(MCH)]
            KH = [persist.tile([128, S], BF16, tag=f"kh{h}", name=f"kh{h}")
                  for h in range(HL)]
            V = [persist.tile([128, HL * VSTRIDE], BF16, tag=f"v{i}",
                              name=f"v{i}")
                 for i in range(SCH)]
            mask_sb = persist.tile([128, SCH], F32, tag="mask")
            nc.sync.dma_start(mask_sb[:], mask[:])
            for h in range(HL):
                # zero the other head's rows once; the real rows are fully
                # overwritten by the K projection chains
                rows = slice(64, 128) if h % 2 == 0 else slice(0, 64)
                nc.vector.memset(KH[h][rows, :].bitcast(U32), 0)
            for i in range(SCH):
                # ones column (col 64 of each head block) for the softmax
                # denominator; cols 0:64 are overwritten by the V chains
                nc.vector.memset(
                    V[i][:].rearrange("p (h j) -> p h j", j=VSTRIDE)
                    [:, :, 64:65], 1.0)

            with tc.tile_pool(name="load", bufs=1) as load, \
                 tc.tile_pool(name="pt", bufs=16) as ptp, \
                 tc.tile_pool(name="ob", bufs=2) as obp, \
                 tc.tile_pool(name="rc", bufs=2) as rcp, \
                 tc.tile_pool(name="scps", bufs=2, space="PSUM") as scp, \
                 tc.tile_pool(name="ctxps", bufs=1, space="PSUM") as ctxp:

                # ---- input DMA on both HWDGE rings (SP + Activation).
                # ring A (SP): wv, xt0, xt2, bv, bq, bk
                # ring B (Act): wq, wk, xt1, xt3
                w_sb = {nm: [None] * DCH for nm in ("q", "k", "v")}
                w_dram = {"q": wq, "k": wk, "v": wv}

                def load_w(nm, dc, eng):
                    t = load.tile([128, DL], BF16, tag=f"w{nm}{dc}",
                                  name=f"w{nm}{dc}")
                    eng.dma_start(t[:], w_dram[nm][dc * 128:(dc + 1) * 128, :])
                    w_sb[nm][dc] = t

                xt_sb = [[None] * DCH for _ in range(4)]  # [col j][dc]

                def load_xt_col(j, eng):
                    for dc in range(DCH):
                        t = load.tile([128, 512], BF16, tag=f"xt{j}_{dc}",
                                      name=f"xt{j}_{dc}")
                        eng.dma_start(
                            t[:],
                            xT[dc * 128:(dc + 1) * 128,
                               j * 512:(j + 1) * 512])
                        xt_sb[j][dc] = t

                load_xt_col(0, nc.sync)
                for dc in range(DCH):
                    load_w("q", dc, nc.scalar)
                for dc in range(DCH):
                    load_w("v", dc, nc.sync)
                for dc in range(DCH):
                    load_w("k", dc, nc.scalar)
                bv_sb = load.tile([1, DL], F32R, tag="bv")
                nc.sync.dma_start(bv_sb[:], bv[:].bitcast(F32R))
                load_xt_col(1, nc.scalar)
                load_xt_col(2, nc.sync)
                load_xt_col(3, nc.scalar)
                bq_sb = load.tile([128, MCH], F32, tag="bq")
                nc.sync.dma_start(bq_sb[:], bq[:])
                bk_sb = load.tile([128, MCH], F32, tag="bk")
                nc.sync.dma_start(bk_sb[:], bk[:])
                ones_row = load.tile([1, 128], F32R, tag="ones_row")
                nc.vector.memset(ones_row[:].bitcast(U32), ONE_F32_BITS)

                # bv broadcast [128, DL] f32 via one K=1 f32r matmul; folded
                # into each V chain's PSUM->SBUF copy as a tensor_tensor add
                bvb_ps = scp.tile([128, DL], F32, tag="c", name="bvb_ps")
                nc.tensor.matmul(bvb_ps[:], ones_row[:], bv_sb[:],
                                 start=True, stop=True)
                bv_bc = load.tile([128, DL], F32, tag="bv_bc")
                nc.vector.tensor_copy(out=bv_bc[:], in_=bvb_ps[:])

                # ---- projection chains (borrowing the scores PSUM slots) ----
                chain_ps = {}

                def qk_chain(wname, mi, q, part=2):
                    # part: 0 = dc 0-2, 1 = dc 3-5 + bias copy, 2 = all
                    key = (wname, mi, q)
                    if part != 1:
                        chain_ps[key] = scp.tile([128, 512], F32, tag="c",
                                                 name="ps_qk")
                    ps = chain_ps[key]
                    dcs = {0: range(3), 1: range(3, DCH), 2: range(DCH)}[part]
                    for dc in dcs:
                        nc.tensor.matmul(
                            ps[:],
                            w_sb[wname][dc][:, mi * 128:(mi + 1) * 128],
                            xt_sb[q][dc][:],
                            start=(dc == 0), stop=(dc == DCH - 1),
                        )
                    if part == 0:
                        return
                    del chain_ps[key]
                    cols = slice(q * 512, (q + 1) * 512)
                    if wname == "q":
                        # bias add fused into the PSUM->SBUF (f32->bf16) copy
                        nc.vector.tensor_scalar_add(
                            QT[mi][:, cols], ps[:], bq_sb[:, mi:mi + 1])
                    else:
                        nc.vector.tensor_scalar_add(
                            KH[2 * mi][0:64, cols], ps[0:64, :],
                            bk_sb[0:64, mi:mi + 1])
                        nc.vector.tensor_scalar_add(
                            KH[2 * mi + 1][64:128, cols], ps[64:128, :],
                            bk_sb[64:128, mi:mi + 1])

                def v_chain(sc, part=2):
                    j, c = divmod(sc, 4)
                    if part != 1:
                        chain_ps[sc] = scp.tile([128, DL], F32, tag="c",
                                                name="ps_v")
                    ps = chain_ps[sc]
                    dcs = {0: range(3), 1: range(3, DCH), 2: range(DCH)}[part]
                    for dc in dcs:
                        nc.tensor.matmul(
                            ps[:],
                            xt_sb[j][dc][:, c * 128:(c + 1) * 128],
                            w_sb["v"][dc][:],
                            start=(dc == 0), stop=(dc == DCH - 1),
                        )
                    if part == 0:
                        return
                    del chain_ps[sc]
                    # V = bf16(ps + bv) with the ones column preserved
                    nc.vector.tensor_tensor(
                        V[sc][:].rearrange(
                            "p (h j) -> p h j", j=VSTRIDE)[:, :, 0:64],
                        ps.rearrange("p (h j) -> p h j", j=64),
                        bv_bc[:].rearrange("p (h j) -> p h j", j=64),
                        ADD)

                # ---- attention with lagged PV emission ----
                s_tiles = {}
                pts = {}
                ctxs = {}

                def alloc_ctx(bi):
                    ctxs[bi] = ctxp.tile([128, QW], F32, tag="ctx",
                                         name="ctx_full")

                def emit_scores(bi, kc):
                    h, qb = blocks[bi]
                    mi = h // 2
                    qo = qb * QW
                    t = scp.tile([128, QW], F32, tag="s", name="s_ps")
                    s_tiles[(bi, kc)] = t
                    for q2 in range(QW // 512):
                        nc.tensor.matmul(
                            t[:, q2 * 512:(q2 + 1) * 512],
                            KH[h][:, kc * 128:(kc + 1) * 128],
                            QT[mi][:, qo + q2 * 512:qo + (q2 + 1) * 512],
                            start=True, stop=True,
                        )

                def emit_pv(g):
                    bi, kc = divmod(g, SCH)
                    h, qb = blocks[bi]
                    if kc == 0:
                        alloc_ctx(bi)
                    ctx_ps = ctxs[bi][0:65, :]
                    pt = pts.pop(g)
                    for q2 in range(QW // 512):
                        nc.tensor.matmul(
                            ctx_ps[:, q2 * 512:(q2 + 1) * 512],
                            V[kc][:, h * VSTRIDE:h * VSTRIDE + 65],
                            pt[:, q2 * 512:(q2 + 1) * 512],
                            start=(kc == 0), stop=(kc == SCH - 1),
                            skip_group_check=True,
                        )

                def emit_tail(bi):
                    # denominator + ctx out of PSUM first (frees the single
                    # ctx slot), then recip -> GPSIMD partition broadcast ->
                    # normalize -> DMA out
                    h, qb = blocks[bi]
                    qo = qb * QW
                    ctx_full = ctxs.pop(bi)
                    den_sb = rcp.tile([1, QW], F32, tag="d", name="den_sb")
                    nc.vector.tensor_copy(out=den_sb[:], in_=ctx_full[64:65, :])
                    ctx_sb = obp.tile([64, QW], F32, tag="cs", name="ctx_sb")
                    nc.vector.tensor_copy(out=ctx_sb[:], in_=ctx_full[0:64, :])
                    recip = rcp.tile([1, QW], F32, tag="r", name="recip")
                    nc.vector.reciprocal(recip[:], den_sb[:])
                    bc_sb = obp.tile([64, QW], F32, tag="bc", name="bc_sb")
                    nc.gpsimd.partition_broadcast(bc_sb[:], recip[:])
                    o_sb = obp.tile([64, QW], F32, tag="o", name="o_sb")
                    nc.vector.tensor_tensor(
                        o_sb[:], ctx_sb[:], bc_sb[:], MULT)
                    nc.sync.dma_start(out[h][:, qo:qo + QW], o_sb[:])

                NG = NBLK * SCH

                # PV(bi,kc) emission iteration: lag ~8 behind the exp; the
                # first six PVs of a block bunch 2-per-iter from +11 so the
                # previous block's ctx drain has released the single slot
                def pv_iter(g):
                    bi, kc = divmod(g, SCH)
                    if bi == NBLK - 1 and kc >= 6:
                        # final block: no woven chains left, collapse the
                        # lag so the kernel drains right behind the last exp
                        return max(g + 2, 16 * bi + 16)
                    if kc < 6:
                        return 16 * bi + 13 + kc // 2
                    return g + 10

                pv_sched = {}
                for g in range(NG):
                    pv_sched.setdefault(pv_iter(g), []).append(g)
                tail_sched = {}
                for bi in range(NBLK):
                    tail_sched[pv_iter(bi * SCH + SCH - 1) + 1] = bi

                # woven chains as half-units, one per iteration, ordered
                # by deadline: K(0,2..3) feed scores kc>=8 (iters 6/10),
                # Q(0,2..3) feed q-block 1 scores (iter 14), V8..15 feed
                # the lag-10 PVs (iter sc+10); mi=1,2 chains (needed from
                # iter 62) trail at one half-unit every other iteration
                chain_sched = {}
                units = []
                for w, q in (("k", 2), ("k", 3)):
                    units += [lambda w=w, q=q: qk_chain(w, 0, q, 0),
                              lambda w=w, q=q: qk_chain(w, 0, q, 1)]
                units += [lambda: v_chain(8, 0), lambda: v_chain(8, 1)]
                for w, q in (("q", 2), ("q", 3)):
                    units += [lambda w=w, q=q: qk_chain(w, 0, q, 0),
                              lambda w=w, q=q: qk_chain(w, 0, q, 1)]
                for sc in range(9, SCH):
                    units += [lambda sc=sc: v_chain(sc, 0),
                              lambda sc=sc: v_chain(sc, 1)]
                for it, u in enumerate(units):
                    chain_sched.setdefault(it, []).append(u)
                late = [(w, mi, q) for mi in (1, 2) for w in ("q", "k")
                        for q in range(4)]
                lu = []
                for w, mi, q in late:
                    lu += [lambda w=w, mi=mi, q=q: qk_chain(w, mi, q, 0),
                           lambda w=w, mi=mi, q=q: qk_chain(w, mi, q, 1)]
                for i, u in enumerate(lu):
                    chain_sched.setdefault(len(units) + 2 + 2 * i, []).append(u)

                # prologue: head-0 Q/K chains + V0..7 under the DMA
                qk_chain("q", 0, 0)
                qk_chain("k", 0, 0)
                for sc in range(4):
                    v_chain(sc)
                qk_chain("q", 0, 1)
                qk_chain("k", 0, 1)
                for sc in range(4, 8):
                    v_chain(sc)
                emit_scores(0, 0)
                emit_scores(0, 1)

                total_iters = pv_iter(NG - 1) + 2
                for it in range(total_iters):
                    if it in tail_sched:
                        emit_tail(tail_sched[it])
                    if it < NG:
                        bi, kc = divmod(it, SCH)
                        pt = ptp.tile([128, QW], BF16, tag="pt", name="pt")
                        pts[it] = pt
                        nc.scalar.activation(
                            pt[:], s_tiles.pop((bi, kc))[:], EXP,
                            bias=mask_sb[:, kc:kc + 1], scale=0.125,
                        )
                        nt = it + 2
                        if nt < NG:
                            emit_scores(*divmod(nt, SCH))
                    for g in pv_sched.get(it, ()):
                        emit_pv(g)
                    for thunk in chain_sched.get(it, ()):
                        thunk()

    nc.compile()
    _cached[("nc", reps)] = nc
    return nc


def shard_inputs(hidden_states, attention_mask, Wq, bq, Wk, bk, Wv, bv):
    """Host-side layout prep (no FLOPs): slice + transpose + bf16 cast."""
    BFD = ml_dtypes.bfloat16
    hidden_states = np.asarray(hidden_states, dtype=np.float32)
    attention_mask = np.asarray(attention_mask, dtype=np.float32)
    Wq, Wk, Wv = (np.asarray(w, dtype=np.float32) for w in (Wq, Wk, Wv))
    bq, bk, bv = (np.asarray(b, dtype=np.float32) for b in (bq, bk, bv))
    in_maps = []
    for c in range(N_CORES):
        b_idx, g = divmod(c, 2)
        cols = slice(g * DL, (g + 1) * DL)
        in_maps.append({
            "xT": np.ascontiguousarray(hidden_states[b_idx].T.astype(BFD)),
            "wq": np.ascontiguousarray(Wq[:, cols].astype(BFD)),
            "wk": np.ascontiguousarray(Wk[:, cols].astype(BFD)),
            "wv": np.ascontiguousarray(Wv[:, cols].astype(BFD)),
            "bq": np.ascontiguousarray(bq[cols].reshape(MCH, 128).T),
            "bk": np.ascontiguousarray(bk[cols].reshape(MCH, 128).T),
            "bv": np.ascontiguousarray(bv[cols].reshape(1, DL)),
            "mask": np.ascontiguousarray(
                attention_mask[b_idx, 0, 0].reshape(SCH, 128).T),
        })
    return in_maps


def assemble_output(results):
    final = np.empty((B, S, D), dtype=np.float32)
    for b_idx in range(B):
        parts = [results[2 * b_idx + g]["out"] for g in range(2)]  # [6, 64, S]
        ctxT = np.concatenate(parts, axis=0)                       # [12, 64, S]
        final[b_idx] = ctxT.transpose(2, 0, 1).reshape(S, D)
    return final


def kernel(**inputs) -> np.ndarray:
    nc = build_program()
    in_maps = shard_inputs(**inputs)
    res = run_bass_kernel_spmd(nc, in_maps, core_ids=list(range(N_CORES)))
    return assemble_output(res.results)


# revision 3
# speedup vs baseline: 1.0149x; 1.0149x over previous
"""BertSelfAttention (B=4, S=2048, D=768, H=12) on 8 Trainium2 NeuronCores.

Sharding: 8 cores = 4 batches x 2 head-groups (6 heads each).

v3: all matmuls bf16 at K=128 contraction (measured ~138ns per 512-col
matmul on HW vs 570-716ns for f32r / K=64 tile modes), so ScalarE's exp
stream (~0.96us per [128,1024] op) is the bottleneck. Host casts to bf16.

Per core, for its (batch b, head-group g):

  QT[mi]  = (Wq_pair^T @ x_b^T)   [128, 2048] bf16, pair-stacked (2 heads)
  KH[h]   = zero-padded per-head K^T: rows (h%2)*64..+64 hold Kh^T, other
            rows zero, so the scores matmul contracts K=128 (fast tile
            mode) while summing only the real 64 dims.
  V[sc]   = x_b @ Wv (+ ones column per head for the softmax denominator)
  scores  S^T[k,q] = KH[h] chunk @ QT[mi]            (PE, K=128 bf16)
  P^T     = exp(S^T * 1/8 + mask[k])                 (ScalarE -> bf16)
  ctx,den = V_aug chunk @ P^T accumulated over kc    (PE, K=128 bf16)
  tail    : denom -> SBUF, 1/denom (DVE), broadcast across partitions
            (GPSIMD partition_broadcast, idle engine), ctx*recip (DVE)

Schedule:
 - input DMA split across both HWDGE rings (SP + Activation) to halve the
   DMA wall; V(0..3) + Q/K chains for head 0 run under the DMA, then the
   attention loop starts (~12us in) and never lets ScalarE idle.
 - the remaining chains (V4..15, rest of Q/K) are woven into the loop's
   PE slack. Because the PE queue is in-order, PV(bi,kc) emission is
   LAGGED ~8 iterations behind exp(bi,kc) (pt ring bufs=16 buffers the
   exp outputs), so a chain stalling on its DMA or a tail stalling on
   the ctx drain never sits in front of the scores the exp stream needs.
 - chains use a dedicated PSUM tag ("c" 2x[128,512]); ctx uses a single
   slot, freed right after the block tail's denominator-copy + drain.

Host side only slices/transposes/casts for layout - all FLOPs on device.
"""

import numpy as np
import ml_dtypes

import concourse.mybir as mybir
import concourse.tile as tile
from concourse import bacc
from concourse.bass_utils import run_bass_kernel_spmd

F32 = mybir.dt.float32
F32R = mybir.dt.float32r
BF16 = mybir.dt.bfloat16
U32 = mybir.dt.uint32
ONE_F32_BITS = 0x3F800000

N_CORES = 8
B, S, D, H = 4, 2048, 768, 12
HL = 6           # heads per core
DH = 64          # head dim
DL = HL * DH     # 384: local output dim
DCH = D // 128   # 6 contraction chunks
MCH = DL // 128  # 3 output-partition chunks (head pairs)
SCH = S // 128   # 16 seq chunks
VSTRIDE = 128    # per-head stride in the augmented-V tile
QW = 1024        # q-block width in the attention loop
QB = S // QW     # 2 q-blocks per head
NBLK = HL * QB   # 12 (head, q-block) attention blocks

_cached = {}


def build_program(reps=1):
    if ("nc", reps) in _cached:
        return _cached[("nc", reps)]
    nc = bacc.Bacc("TRN2", target_bir_lowering=False, debug=False, num_devices=1)
    xT = nc.dram_tensor("xT", [D, S], BF16, kind="ExternalInput").ap()
    wq = nc.dram_tensor("wq", [D, DL], BF16, kind="ExternalInput").ap()
    wk = nc.dram_tensor("wk", [D, DL], BF16, kind="ExternalInput").ap()
    wv = nc.dram_tensor("wv", [D, DL], BF16, kind="ExternalInput").ap()
    bq = nc.dram_tensor("bq", [128, MCH], F32, kind="ExternalInput").ap()
    bk = nc.dram_tensor("bk", [128, MCH], F32, kind="ExternalInput").ap()
    bv = nc.dram_tensor("bv", [1, DL], F32, kind="ExternalInput").ap()
    mask = nc.dram_tensor("mask", [128, SCH], F32, kind="ExternalInput").ap()
    out = nc.dram_tensor("out", [HL, DH, S], F32, kind="ExternalOutput").ap()

    EXP = mybir.ActivationFunctionType.Exp
    MULT = mybir.AluOpType.mult
    ADD = mybir.AluOpType.add

    blocks = [(h, qb) for h in range(HL) for qb in range(QB)]

    with tile.TileContext(nc) as tc:
      for _rep in range(reps):
        with tc.tile_pool(name="persist", bufs=1) as persist:
            QT = [persist.tile([128, S], BF16, tag=f"qt{i}", name=f"qt{i}")
                  for i in range(MCH)]
            KH = [persist.tile([128, S], BF16, tag=f"kh{h}", name=f"kh{h}")
                  for h in range(HL)]
            V = [persist.tile([128, HL * VSTRIDE], BF16, tag=f"v{i}",
                              name=f"v{i}")
                 for i in range(SCH)]
            mask_sb = persist.tile([128, SCH], F32, tag="mask")
            nc.sync.dma_start(mask_sb[:], mask[:])
            for h in range(HL):
                # zero the other head's rows once; the real rows are fully
                # overwritten by the K projection chains
                rows = slice(64, 128) if h % 2 == 0 else slice(0, 64)
                nc.vector.memset(KH[h][rows, :].bitcast(U32), 0)
            for i in range(SCH):
                # ones column (col 64 of each head block) for the softmax
                # denominator; cols 0:64 are overwritten by the V chains
                nc.vector.memset(
                    V[i][:].rearrange("p (h j) -> p h j", j=VSTRIDE)
                    [:, :, 64:65], 1.0)

            with tc.tile_pool(name="load", bufs=1) as load, \
                 tc.tile_pool(name="pt", bufs=16) as ptp, \
                 tc.tile_pool(name="ob", bufs=2) as obp, \
                 tc.tile_pool(name="rc", bufs=2) as rcp, \
                 tc.tile_pool(name="scps", bufs=2, space="PSUM") as scp, \
                 tc.tile_pool(name="ctxps", bufs=1, space="PSUM") as ctxp:

                # ---- input DMA on both HWDGE rings (SP + Activation).
                # ring A (SP): wv, xt0, xt2, bv, bq, bk
                # ring B (Act): wq, wk, xt1, xt3
                w_sb = {nm: [None] * DCH for nm in ("q", "k", "v")}
                w_dram = {"q": wq, "k": wk, "v": wv}

                def load_w(nm, dc, eng):
                    t = load.tile([128, DL], BF16, tag=f"w{nm}{dc}",
                                  name=f"w{nm}{dc}")
                    eng.dma_start(t[:], w_dram[nm][dc * 128:(dc + 1) * 128, :])
                    w_sb[nm][dc] = t

                xt_sb = [[None] * DCH for _ in range(4)]  # [col j][dc]

                def load_xt_col(j, eng):
                    for dc in range(DCH):
                        t = load.tile([128, 512], BF16, tag=f"xt{j}_{dc}",
                                      name=f"xt{j}_{dc}")
                        eng.dma_start(
                            t[:],
                            xT[dc * 128:(dc + 1) * 128,
                               j * 512:(j + 1) * 512])
                        xt_sb[j][dc] = t

                bq_sb = load.tile([128, MCH], F32, tag="bq")
                nc.sync.dma_start(bq_sb[:], bq[:])
                bk_sb = load.tile([128, MCH], F32, tag="bk")
                nc.sync.dma_start(bk_sb[:], bk[:])
                load_xt_col(0, nc.sync)
                for dc in range(DCH):
                    load_w("q", dc, nc.scalar)
                for dc in range(DCH):
                    load_w("v", dc, nc.sync)
                for dc in range(DCH):
                    load_w("k", dc, nc.scalar)
                bv_sb = load.tile([1, DL], F32R, tag="bv")
                nc.sync.dma_start(bv_sb[:], bv[:].bitcast(F32R))
                load_xt_col(1, nc.scalar)
                load_xt_col(2, nc.sync)
                load_xt_col(3, nc.scalar)
                ones_row = load.tile([1, 128], F32R, tag="ones_row")
                nc.vector.memset(ones_row[:].bitcast(U32), ONE_F32_BITS)

                # bv broadcast [128, DL] f32 via one K=1 f32r matmul; folded
                # into each V chain's PSUM->SBUF copy as a tensor_tensor add
                bvb_ps = scp.tile([128, DL], F32, tag="c", name="bvb_ps")
                nc.tensor.matmul(bvb_ps[:], ones_row[:], bv_sb[:],
                                 start=True, stop=True)
                bv_bc = load.tile([128, DL], F32, tag="bv_bc")
                nc.vector.tensor_copy(out=bv_bc[:], in_=bvb_ps[:])

                # ---- projection chains (borrowing the scores PSUM slots) ----
                chain_ps = {}

                def qk_chain(wname, mi, q, part=2):
                    # part: 0 = dc 0-2, 1 = dc 3-5 + bias copy, 2 = all
                    key = (wname, mi, q)
                    if part != 1:
                        chain_ps[key] = scp.tile([128, 512], F32, tag="c",
                                                 name="ps_qk")
                    ps = chain_ps[key]
                    dcs = {0: range(3), 1: range(3, DCH), 2: range(DCH)}[part]
                    for dc in dcs:
                        nc.tensor.matmul(
                            ps[:],
                            w_sb[wname][dc][:, mi * 128:(mi + 1) * 128],
                            xt_sb[q][dc][:],
                            start=(dc == 0), stop=(dc == DCH - 1),
                        )
                    if part == 0:
                        return
                    del chain_ps[key]
                    cols = slice(q * 512, (q + 1) * 512)
                    if wname == "q":
                        # bias add fused into the PSUM->SBUF (f32->bf16) copy
                        nc.vector.tensor_scalar_add(
                            QT[mi][:, cols], ps[:], bq_sb[:, mi:mi + 1])
                    else:
                        nc.vector.tensor_scalar_add(
                            KH[2 * mi][0:64, cols], ps[0:64, :],
                            bk_sb[0:64, mi:mi + 1])
                        nc.vector.tensor_scalar_add(
                            KH[2 * mi + 1][64:128, cols], ps[64:128, :],
                            bk_sb[64:128, mi:mi + 1])

                def v_chain(sc, part=2):
                    j, c = divmod(sc, 4)
                    if part != 1:
                        chain_ps[sc] = scp.tile([128, DL], F32, tag="c",
                                                name="ps_v")
                    ps = chain_ps[sc]
                    dcs = {0: range(3), 1: range(3, DCH), 2: range(DCH)}[part]
                    for dc in dcs:
                        nc.tensor.matmul(
                            ps[:],
                            xt_sb[j][dc][:, c * 128:(c + 1) * 128],
                            w_sb["v"][dc][:],
                            start=(dc == 0), stop=(dc == DCH - 1),
                        )
                    if part == 0:
                        return
                    del chain_ps[sc]
                    # V = bf16(ps + bv) with the ones column preserved
                    nc.vector.tensor_tensor(
                        V[sc][:].rearrange(
                            "p (h j) -> p h j", j=VSTRIDE)[:, :, 0:64],
                        ps.rearrange("p (h j) -> p h j", j=64),
                        bv_bc[:].rearrange("p (h j) -> p h j", j=64),
                        ADD)

                # ---- attention with lagged PV emission ----
                s_tiles = {}
                pts = {}
                ctxs = {}

                def alloc_ctx(bi):
                    ctxs[bi] = ctxp.tile([128, QW], F32, tag="ctx",
                                         name="ctx_full")

                def emit_scores(bi, kc):
                    h, qb = blocks[bi]
                    mi = h // 2
                    qo = qb * QW
                    t = scp.tile([128, QW], F32, tag="s", name="s_ps")
                    s_tiles[(bi, kc)] = t
                    for q2 in range(QW // 512):
                        nc.tensor.matmul(
                            t[:, q2 * 512:(q2 + 1) * 512],
                            KH[h][:, kc * 128:(kc + 1) * 128],
                            QT[mi][:, qo + q2 * 512:qo + (q2 + 1) * 512],
                            start=True, stop=True,
                        )

                def emit_pv(g):
                    bi, kc = divmod(g, SCH)
                    h, qb = blocks[bi]
                    if kc == 0:
                        alloc_ctx(bi)
                    ctx_ps = ctxs[bi][0:65, :]
                    pt = pts.pop(g)
                    for q2 in range(QW // 512):
                        nc.tensor.matmul(
                            ctx_ps[:, q2 * 512:(q2 + 1) * 512],
                            V[kc][:, h * VSTRIDE:h * VSTRIDE + 65],
                            pt[:, q2 * 512:(q2 + 1) * 512],
                            start=(kc == 0), stop=(kc == SCH - 1),
                            skip_group_check=True,
                        )

                def emit_tail(bi, split=False):
                    # denominator + ctx out of PSUM first (frees the single
                    # ctx slot), then recip -> GPSIMD partition broadcast ->
                    # normalize -> DMA out. split=True pipelines the tail in
                    # two column halves (used for the last block's drain).
                    h, qb = blocks[bi]
                    qo = qb * QW
                    ctx_full = ctxs.pop(bi)
                    den_sb = rcp.tile([1, QW], F32, tag="d", name="den_sb")
                    nc.vector.tensor_copy(out=den_sb[:], in_=ctx_full[64:65, :])
                    ctx_sb = obp.tile([64, QW], F32, tag="cs", name="ctx_sb")
                    bc_sb = obp.tile([64, QW], F32, tag="bc", name="bc_sb")
                    o_sb = obp.tile([64, QW], F32, tag="o", name="o_sb")
                    recip = rcp.tile([1, QW], F32, tag="r", name="recip")
                    halves = ((slice(0, 512), slice(512, QW)) if split
                              else (slice(0, QW),))
                    for cs in halves:
                        nc.vector.tensor_copy(out=ctx_sb[:, cs],
                                              in_=ctx_full[0:64, cs])
                        nc.vector.reciprocal(recip[:, cs], den_sb[:, cs])
                        nc.gpsimd.partition_broadcast(bc_sb[:, cs],
                                                      recip[:, cs])
                        nc.vector.tensor_tensor(
                            o_sb[:, cs], ctx_sb[:, cs], bc_sb[:, cs], MULT)
                        nc.sync.dma_start(out[h][:, qo + cs.start:
                                                 qo + cs.stop], o_sb[:, cs])

                NG = NBLK * SCH

                # PV(bi,kc) emission iteration: lag ~8 behind the exp; the
                # first six PVs of a block bunch 2-per-iter from +11 so the
                # previous block's ctx drain has released the single slot
                def pv_iter(g):
                    bi, kc = divmod(g, SCH)
                    if bi == NBLK - 1 and kc >= 6:
                        # final block: no woven chains left, collapse the
                        # lag so the kernel drains right behind the last exp
                        return max(g + 2, 16 * bi + 16)
                    if kc < 6:
                        return 16 * bi + 13 + kc // 2
                    return g + 10

                pv_sched = {}
                for g in range(NG):
                    pv_sched.setdefault(pv_iter(g), []).append(g)
                tail_sched = {}
                for bi in range(NBLK):
                    tail_sched[pv_iter(bi * SCH + SCH - 1) + 1] = bi

                # woven chains as half-units, one per iteration, ordered
                # by deadline: K(0,2..3) feed scores kc>=8 (iters 6/10),
                # Q(0,2..3) feed q-block 1 scores (iter 14), V8..15 feed
                # the lag-10 PVs (iter sc+10); mi=1,2 chains (needed from
                # iter 62) trail at one half-unit every other iteration
                chain_sched = {}
                units = []
                for w, q in (("k", 2), ("k", 3)):
                    units += [lambda w=w, q=q: qk_chain(w, 0, q, 0),
                              lambda w=w, q=q: qk_chain(w, 0, q, 1)]
                units += [lambda: v_chain(8, 0), lambda: v_chain(8, 1)]
                for w, q in (("q", 2), ("q", 3)):
                    units += [lambda w=w, q=q: qk_chain(w, 0, q, 0),
                              lambda w=w, q=q: qk_chain(w, 0, q, 1)]
                for sc in range(9, SCH):
                    units += [lambda sc=sc: v_chain(sc, 0),
                              lambda sc=sc: v_chain(sc, 1)]
                for it, u in enumerate(units):
                    chain_sched.setdefault(it, []).append(u)
                late = [(w, mi, q) for mi in (1, 2) for w in ("q", "k")
                        for q in range(4)]
                lu = []
                for w, mi, q in late:
                    lu += [lambda w=w, mi=mi, q=q: qk_chain(w, mi, q, 0),
                           lambda w=w, mi=mi, q=q: qk_chain(w, mi, q, 1)]
                for i, u in enumerate(lu):
                    chain_sched.setdefault(len(units) + 2 + 2 * i, []).append(u)

                # prologue: head-0 Q/K chains + V0..7 under the DMA
                qk_chain("q", 0, 0)
                qk_chain("k", 0, 0)
                for sc in range(4):
                    v_chain(sc)
                qk_chain("q", 0, 1)
                qk_chain("k", 0, 1)
                for sc in range(4, 8):
                    v_chain(sc)
                emit_scores(0, 0)
                emit_scores(0, 1)

                total_iters = pv_iter(NG - 1) + 2
                for it in range(total_iters):
                    if it in tail_sched:
                        emit_tail(tail_sched[it],
                                  split=(tail_sched[it] == NBLK - 1))
                    if it < NG:
                        bi, kc = divmod(it, SCH)
                        pt = ptp.tile([128, QW], BF16, tag="pt", name="pt")
                        pts[it] = pt
                        nc.scalar.activation(
                            pt[:], s_tiles.pop((bi, kc))[:], EXP,
                            bias=mask_sb[:, kc:kc + 1], scale=0.125,
                        )
                        nt = it + 2
                        if nt < NG:
                            emit_scores(*divmod(nt, SCH))
                    for g in pv_sched.get(it, ()):
                        emit_pv(g)
                    for thunk in chain_sched.get(it, ()):
                        thunk()

    nc.compile()
    _cached[("nc", reps)] = nc
    return nc


def shard_inputs(hidden_states, attention_mask, Wq, bq, Wk, bk, Wv, bv):
    """Host-side layout prep (no FLOPs): slice + transpose + bf16 cast."""
    BFD = ml_dtypes.bfloat16
    hidden_states = np.asarray(hidden_states, dtype=np.float32)
    attention_mask = np.asarray(attention_mask, dtype=np.float32)
    Wq, Wk, Wv = (np.asarray(w, dtype=np.float32) for w in (Wq, Wk, Wv))
    bq, bk, bv = (np.asarray(b, dtype=np.float32) for b in (bq, bk, bv))
    in_maps = []
    for c in range(N_CORES):
        b_idx, g = divmod(c, 2)
        cols = slice(g * DL, (g + 1) * DL)
        in_maps.append({
            "xT": np.ascontiguousarray(hidden_states[b_idx].T.astype(BFD)),
            "wq": np.ascontiguousarray(Wq[:, cols].astype(BFD)),
            "wk": np.ascontiguousarray(Wk[:, cols].astype(BFD)),
            "wv": np.ascontiguousarray(Wv[:, cols].astype(BFD)),
            "bq": np.ascontiguousarray(bq[cols].reshape(MCH, 128).T),
            "bk": np.ascontiguousarray(bk[cols].reshape(MCH, 128).T),
            "bv": np.ascontiguousarray(bv[cols].reshape(1, DL)),
            "mask": np.ascontiguousarray(
                attention_mask[b_idx, 0, 0].reshape(SCH, 128).T),
        })
    return in_maps


def assemble_output(results):
    final = np.empty((B, S, D), dtype=np.float32)
    for b_idx in range(B):
        parts = [results[2 * b_idx + g]["out"] for g in range(2)]  # [6, 64, S]
        ctxT = np.concatenate(parts, axis=0)                       # [12, 64, S]
        final[b_idx] = ctxT.transpose(2, 0, 1).reshape(S, D)
    return final


def kernel(**inputs) -> np.ndarray:
    nc = build_program()
    in_maps = shard_inputs(**inputs)
    res = run_bass_kernel_spmd(nc, in_maps, core_ids=list(range(N_CORES)))
    return assemble_output(res.results)


# revision 4
# speedup vs baseline: 1.0289x; 1.0138x over previous
"""BertSelfAttention (B=4, S=2048, D=768, H=12) on 8 Trainium2 NeuronCores.

Sharding: 8 cores = 4 batches x 2 head-groups (6 heads each).

v3: all matmuls bf16 at K=128 contraction (measured ~138ns per 512-col
matmul on HW vs 570-716ns for f32r / K=64 tile modes), so ScalarE's exp
stream (~0.96us per [128,1024] op) is the bottleneck. Host casts to bf16.

Per core, for its (batch b, head-group g):

  QT[mi]  = (Wq_pair^T @ x_b^T)   [128, 2048] bf16, pair-stacked (2 heads)
  KH[h]   = zero-padded per-head K^T: rows (h%2)*64..+64 hold Kh^T, other
            rows zero, so the scores matmul contracts K=128 (fast tile
            mode) while summing only the real 64 dims.
  V[sc]   = x_b @ Wv (+ ones column per head for the softmax denominator)
  scores  S^T[k,q] = KH[h] chunk @ QT[mi]            (PE, K=128 bf16)
  P^T     = exp(S^T * 1/8 + mask[k])                 (ScalarE -> bf16)
  ctx,den = V_aug chunk @ P^T accumulated over kc    (PE, K=128 bf16)
  tail    : denom -> SBUF, 1/denom (DVE), broadcast across partitions
            (GPSIMD partition_broadcast, idle engine), ctx*recip (DVE)

Schedule:
 - input DMA split across both HWDGE rings (SP + Activation) to halve the
   DMA wall; V(0..3) + Q/K chains for head 0 run under the DMA, then the
   attention loop starts (~12us in) and never lets ScalarE idle.
 - the remaining chains (V4..15, rest of Q/K) are woven into the loop's
   PE slack. Because the PE queue is in-order, PV(bi,kc) emission is
   LAGGED ~8 iterations behind exp(bi,kc) (pt ring bufs=16 buffers the
   exp outputs), so a chain stalling on its DMA or a tail stalling on
   the ctx drain never sits in front of the scores the exp stream needs.
 - chains use a dedicated PSUM tag ("c" 2x[128,512]); ctx uses a single
   slot, freed right after the block tail's denominator-copy + drain.

Host side only slices/transposes/casts for layout - all FLOPs on device.
"""

import numpy as np
import ml_dtypes

import concourse.mybir as mybir
import concourse.tile as tile
from concourse import bacc
from concourse.bass_utils import run_bass_kernel_spmd

F32 = mybir.dt.float32
F32R = mybir.dt.float32r
BF16 = mybir.dt.bfloat16
U32 = mybir.dt.uint32
ONE_F32_BITS = 0x3F800000

N_CORES = 8
B, S, D, H = 4, 2048, 768, 12
HL = 6           # heads per core
DH = 64          # head dim
DL = HL * DH     # 384: local output dim
DCH = D // 128   # 6 contraction chunks
MCH = DL // 128  # 3 output-partition chunks (head pairs)
SCH = S // 128   # 16 seq chunks
VSTRIDE = 128    # per-head stride in the augmented-V tile
QW = 1024        # q-block width in the attention loop
QB = S // QW     # 2 q-blocks per head
NBLK = HL * QB   # 12 (head, q-block) attention blocks

_cached = {}


def build_program(reps=1):
    if ("nc", reps) in _cached:
        return _cached[("nc", reps)]
    nc = bacc.Bacc("TRN2", target_bir_lowering=False, debug=False, num_devices=1)
    xT = nc.dram_tensor("xT", [D, S], BF16, kind="ExternalInput").ap()
    wq = nc.dram_tensor("wq", [D, DL], BF16, kind="ExternalInput").ap()
    wk = nc.dram_tensor("wk", [D, DL], BF16, kind="ExternalInput").ap()
    wv = nc.dram_tensor("wv", [D, DL], BF16, kind="ExternalInput").ap()
    bq = nc.dram_tensor("bq", [128, MCH], F32, kind="ExternalInput").ap()
    bk = nc.dram_tensor("bk", [128, MCH], F32, kind="ExternalInput").ap()
    bv = nc.dram_tensor("bv", [1, DL], F32, kind="ExternalInput").ap()
    mask = nc.dram_tensor("mask", [128, SCH], F32, kind="ExternalInput").ap()
    out = nc.dram_tensor("out", [HL, DH, S], F32, kind="ExternalOutput").ap()

    EXP = mybir.ActivationFunctionType.Exp
    MULT = mybir.AluOpType.mult
    ADD = mybir.AluOpType.add

    blocks = [(h, qb) for h in range(HL) for qb in range(QB)]

    with tile.TileContext(nc) as tc:
      for _rep in range(reps):
        with tc.tile_pool(name="persist", bufs=1) as persist:
            QT = [persist.tile([128, S], BF16, tag=f"qt{i}", name=f"qt{i}")
                  for i in range(MCH)]
            KH = [persist.tile([128, S], BF16, tag=f"kh{h}", name=f"kh{h}")
                  for h in range(HL)]
            V = [persist.tile([128, HL * VSTRIDE], BF16, tag=f"v{i}",
                              name=f"v{i}")
                 for i in range(SCH)]
            mask_sb = persist.tile([128, SCH], F32, tag="mask")
            nc.sync.dma_start(mask_sb[:], mask[:])
            for h in range(HL):
                # zero the other head's rows once; the real rows are fully
                # overwritten by the K projection chains
                rows = slice(64, 128) if h % 2 == 0 else slice(0, 64)
                nc.vector.memset(KH[h][rows, :].bitcast(U32), 0)
            for i in range(SCH):
                # ones column (col 64 of each head block) for the softmax
                # denominator; cols 0:64 are overwritten by the V chains
                nc.vector.memset(
                    V[i][:].rearrange("p (h j) -> p h j", j=VSTRIDE)
                    [:, :, 64:65], 1.0)

            with tc.tile_pool(name="load", bufs=1) as load, \
                 tc.tile_pool(name="pt", bufs=16) as ptp, \
                 tc.tile_pool(name="ob", bufs=2) as obp, \
                 tc.tile_pool(name="rc", bufs=2) as rcp, \
                 tc.tile_pool(name="scps", bufs=2, space="PSUM") as scp, \
                 tc.tile_pool(name="ctxps", bufs=1, space="PSUM") as ctxp:

                # ---- input DMA on both HWDGE rings (SP + Activation).
                # ring A (SP): wv, xt0, xt2, bv, bq, bk
                # ring B (Act): wq, wk, xt1, xt3
                w_sb = {nm: [None] * DCH for nm in ("q", "k", "v")}
                w_dram = {"q": wq, "k": wk, "v": wv}

                def load_w(nm, dc, eng):
                    t = load.tile([128, DL], BF16, tag=f"w{nm}{dc}",
                                  name=f"w{nm}{dc}")
                    eng.dma_start(t[:], w_dram[nm][dc * 128:(dc + 1) * 128, :])
                    w_sb[nm][dc] = t

                xt_sb = [[None] * DCH for _ in range(4)]  # [col j][dc]

                def load_xt_col(j, eng):
                    for dc in range(DCH):
                        t = load.tile([128, 512], BF16, tag=f"xt{j}_{dc}",
                                      name=f"xt{j}_{dc}")
                        eng.dma_start(
                            t[:],
                            xT[dc * 128:(dc + 1) * 128,
                               j * 512:(j + 1) * 512])
                        xt_sb[j][dc] = t

                bq_sb = load.tile([128, MCH], F32, tag="bq")
                nc.sync.dma_start(bq_sb[:], bq[:])
                bk_sb = load.tile([128, MCH], F32, tag="bk")
                nc.sync.dma_start(bk_sb[:], bk[:])
                load_xt_col(0, nc.sync)
                for dc in range(DCH):
                    load_w("q", dc, nc.scalar)
                for dc in range(DCH):
                    load_w("v", dc, nc.sync)
                for dc in range(DCH):
                    load_w("k", dc, nc.scalar)
                bv_sb = load.tile([1, DL], F32R, tag="bv")
                nc.sync.dma_start(bv_sb[:], bv[:].bitcast(F32R))
                load_xt_col(1, nc.scalar)
                load_xt_col(2, nc.sync)
                load_xt_col(3, nc.scalar)
                ones_row = load.tile([1, 128], F32R, tag="ones_row")
                nc.vector.memset(ones_row[:].bitcast(U32), ONE_F32_BITS)

                # bv broadcast [128, DL] f32 via one K=1 f32r matmul; folded
                # into each V chain's PSUM->SBUF copy as a tensor_tensor add
                bvb_ps = scp.tile([128, DL], F32, tag="c", name="bvb_ps")
                nc.tensor.matmul(bvb_ps[:], ones_row[:], bv_sb[:],
                                 start=True, stop=True)
                bv_bc = load.tile([128, DL], F32, tag="bv_bc")
                nc.vector.tensor_copy(out=bv_bc[:], in_=bvb_ps[:])

                # ---- projection chains (borrowing the scores PSUM slots) ----
                chain_ps = {}

                def qk_chain(wname, mi, q, part=2):
                    # part: 0 = dc 0-2, 1 = dc 3-5 + bias copy, 2 = all
                    key = (wname, mi, q)
                    if part != 1:
                        chain_ps[key] = scp.tile([128, 512], F32, tag="c",
                                                 name="ps_qk")
                    ps = chain_ps[key]
                    dcs = {0: range(3), 1: range(3, DCH), 2: range(DCH)}[part]
                    for dc in dcs:
                        nc.tensor.matmul(
                            ps[:],
                            w_sb[wname][dc][:, mi * 128:(mi + 1) * 128],
                            xt_sb[q][dc][:],
                            start=(dc == 0), stop=(dc == DCH - 1),
                        )
                    if part == 0:
                        return
                    del chain_ps[key]
                    cols = slice(q * 512, (q + 1) * 512)
                    if wname == "q":
                        # bias add fused into the PSUM->SBUF (f32->bf16) copy
                        nc.vector.tensor_scalar_add(
                            QT[mi][:, cols], ps[:], bq_sb[:, mi:mi + 1])
                    else:
                        nc.vector.tensor_scalar_add(
                            KH[2 * mi][0:64, cols], ps[0:64, :],
                            bk_sb[0:64, mi:mi + 1])
                        nc.vector.tensor_scalar_add(
                            KH[2 * mi + 1][64:128, cols], ps[64:128, :],
                            bk_sb[64:128, mi:mi + 1])

                def v_chain(sc, part=2):
                    j, c = divmod(sc, 4)
                    if part != 1:
                        chain_ps[sc] = scp.tile([128, DL], F32, tag="c",
                                                name="ps_v")
                    ps = chain_ps[sc]
                    dcs = {0: range(3), 1: range(3, DCH), 2: range(DCH)}[part]
                    for dc in dcs:
                        nc.tensor.matmul(
                            ps[:],
                            xt_sb[j][dc][:, c * 128:(c + 1) * 128],
                            w_sb["v"][dc][:],
                            start=(dc == 0), stop=(dc == DCH - 1),
                        )
                    if part == 0:
                        return
                    del chain_ps[sc]
                    # V = bf16(ps + bv) with the ones column preserved
                    nc.vector.tensor_tensor(
                        V[sc][:].rearrange(
                            "p (h j) -> p h j", j=VSTRIDE)[:, :, 0:64],
                        ps.rearrange("p (h j) -> p h j", j=64),
                        bv_bc[:].rearrange("p (h j) -> p h j", j=64),
                        ADD)

                # ---- attention with lagged PV emission ----
                s_tiles = {}
                pts = {}
                ctxs = {}

                def alloc_ctx(bi):
                    ctxs[bi] = ctxp.tile([128, QW], F32, tag="ctx",
                                         name="ctx_full")

                def emit_scores(bi, kc):
                    h, qb = blocks[bi]
                    mi = h // 2
                    qo = qb * QW
                    t = scp.tile([128, QW], F32, tag="s", name="s_ps")
                    s_tiles[(bi, kc)] = t
                    for q2 in range(QW // 512):
                        nc.tensor.matmul(
                            t[:, q2 * 512:(q2 + 1) * 512],
                            KH[h][:, kc * 128:(kc + 1) * 128],
                            QT[mi][:, qo + q2 * 512:qo + (q2 + 1) * 512],
                            start=True, stop=True,
                        )

                def emit_pv(g):
                    bi, kc = divmod(g, SCH)
                    h, qb = blocks[bi]
                    if kc == 0:
                        alloc_ctx(bi)
                    ctx_ps = ctxs[bi][0:65, :]
                    pt = pts.pop(g)
                    for q2 in range(QW // 512):
                        nc.tensor.matmul(
                            ctx_ps[:, q2 * 512:(q2 + 1) * 512],
                            V[kc][:, h * VSTRIDE:h * VSTRIDE + 65],
                            pt[:, q2 * 512:(q2 + 1) * 512],
                            start=(kc == 0), stop=(kc == SCH - 1),
                            skip_group_check=True,
                        )

                def emit_tail(bi, split=False):
                    # denominator + ctx out of PSUM first (frees the single
                    # ctx slot), then recip -> GPSIMD partition broadcast ->
                    # normalize -> DMA out. split=True pipelines the tail in
                    # two column halves (used for the last block's drain).
                    h, qb = blocks[bi]
                    qo = qb * QW
                    ctx_full = ctxs.pop(bi)
                    den_sb = rcp.tile([1, QW], F32, tag="d", name="den_sb")
                    nc.vector.tensor_copy(out=den_sb[:], in_=ctx_full[64:65, :])
                    ctx_sb = obp.tile([64, QW], F32, tag="cs", name="ctx_sb")
                    bc_sb = obp.tile([64, QW], F32, tag="bc", name="bc_sb")
                    o_sb = obp.tile([64, QW], F32, tag="o", name="o_sb")
                    recip = rcp.tile([1, QW], F32, tag="r", name="recip")
                    halves = ((slice(0, 512), slice(512, QW)) if split
                              else (slice(0, QW),))
                    for cs in halves:
                        nc.vector.tensor_copy(out=ctx_sb[:, cs],
                                              in_=ctx_full[0:64, cs])
                        nc.vector.reciprocal(recip[:, cs], den_sb[:, cs])
                        nc.gpsimd.partition_broadcast(bc_sb[:, cs],
                                                      recip[:, cs])
                        nc.vector.tensor_tensor(
                            o_sb[:, cs], ctx_sb[:, cs], bc_sb[:, cs], MULT)
                        nc.sync.dma_start(out[h][:, qo + cs.start:
                                                 qo + cs.stop], o_sb[:, cs])

                NG = NBLK * SCH

                # PV(bi,kc) emission iteration: lag ~8 behind the exp; the
                # first six PVs of a block bunch 2-per-iter from +11 so the
                # previous block's ctx drain has released the single slot
                def pv_iter(g):
                    bi, kc = divmod(g, SCH)
                    if bi == NBLK - 1 and kc >= 6:
                        # final block: no woven chains left, collapse the
                        # lag so the kernel drains right behind the last exp
                        return max(g + 2, 16 * bi + 16)
                    if kc < 6:
                        return 16 * bi + 13 + kc // 2
                    return g + 10

                pv_sched = {}
                for g in range(NG):
                    pv_sched.setdefault(pv_iter(g), []).append(g)
                tail_sched = {}
                for bi in range(NBLK):
                    tail_sched[pv_iter(bi * SCH + SCH - 1) + 1] = bi

                # woven chains as half-units, one per iteration, ordered
                # by deadline: K(0,2..3) feed scores kc>=8 (iters 6/10),
                # Q(0,2..3) feed q-block 1 scores (iter 14), V8..15 feed
                # the lag-10 PVs (iter sc+10); mi=1,2 chains (needed from
                # iter 62) trail at one half-unit every other iteration
                chain_sched = {}
                units = []
                for w, q in (("k", 2), ("k", 3)):
                    units += [lambda w=w, q=q: qk_chain(w, 0, q, 0),
                              lambda w=w, q=q: qk_chain(w, 0, q, 1)]
                units += [lambda: v_chain(8, 0), lambda: v_chain(8, 1)]
                for w, q in (("q", 2), ("q", 3)):
                    units += [lambda w=w, q=q: qk_chain(w, 0, q, 0),
                              lambda w=w, q=q: qk_chain(w, 0, q, 1)]
                for sc in range(9, SCH):
                    units += [lambda sc=sc: v_chain(sc, 0),
                              lambda sc=sc: v_chain(sc, 1)]
                for it, u in enumerate(units):
                    chain_sched.setdefault(it, []).append(u)
                late = [(w, mi, q) for mi in (1, 2) for w in ("q", "k")
                        for q in range(4)]
                lu = []
                for w, mi, q in late:
                    lu += [lambda w=w, mi=mi, q=q: qk_chain(w, mi, q, 0),
                           lambda w=w, mi=mi, q=q: qk_chain(w, mi, q, 1)]
                for i, u in enumerate(lu):
                    chain_sched.setdefault(len(units) + 2 + 3 * i, []).append(u)

                # prologue: head-0 Q/K chains + V0..7 under the DMA
                qk_chain("q", 0, 0)
                qk_chain("k", 0, 0)
                for sc in range(4):
                    v_chain(sc)
                qk_chain("q", 0, 1)
                qk_chain("k", 0, 1)
                for sc in range(4, 8):
                    v_chain(sc)
                emit_scores(0, 0)
                emit_scores(0, 1)

                total_iters = pv_iter(NG - 1) + 2
                for it in range(total_iters):
                    if it in tail_sched:
                        emit_tail(tail_sched[it],
                                  split=(tail_sched[it] == NBLK - 1))
                    if it < NG:
                        bi, kc = divmod(it, SCH)
                        pt = ptp.tile([128, QW], BF16, tag="pt", name="pt")
                        pts[it] = pt
                        nc.scalar.activation(
                            pt[:], s_tiles.pop((bi, kc))[:], EXP,
                            bias=mask_sb[:, kc:kc + 1], scale=0.125,
                        )
                        nt = it + 2
                        if nt < NG:
                            emit_scores(*divmod(nt, SCH))
                    for g in pv_sched.get(it, ()):
                        emit_pv(g)
                    for thunk in chain_sched.get(it, ()):
                        thunk()

    nc.compile()
    _cached[("nc", reps)] = nc
    return nc


def shard_inputs(hidden_states, attention_mask, Wq, bq, Wk, bk, Wv, bv):
    """Host-side layout prep (no FLOPs): slice + transpose + bf16 cast."""
    BFD = ml_dtypes.bfloat16
    hidden_states = np.asarray(hidden_states, dtype=np.float32)
    attention_mask = np.asarray(attention_mask, dtype=np.float32)
    Wq, Wk, Wv = (np.asarray(w, dtype=np.float32) for w in (Wq, Wk, Wv))
    bq, bk, bv = (np.asarray(b, dtype=np.float32) for b in (bq, bk, bv))
    in_maps = []
    for c in range(N_CORES):
        b_idx, g = divmod(c, 2)
        cols = slice(g * DL, (g + 1) * DL)
        in_maps.append({
            "xT": np.ascontiguousarray(hidden_states[b_idx].T.astype(BFD)),
            "wq": np.ascontiguousarray(Wq[:, cols].astype(BFD)),
            "wk": np.ascontiguousarray(Wk[:, cols].astype(BFD)),
            "wv": np.ascontiguousarray(Wv[:, cols].astype(BFD)),
            "bq": np.ascontiguousarray(bq[cols].reshape(MCH, 128).T),
            "bk": np.ascontiguousarray(bk[cols].reshape(MCH, 128).T),
            "bv": np.ascontiguousarray(bv[cols].reshape(1, DL)),
            "mask": np.ascontiguousarray(
                attention_mask[b_idx, 0, 0].reshape(SCH, 128).T),
        })
    return in_maps


def assemble_output(results):
    final = np.empty((B, S, D), dtype=np.float32)
    for b_idx in range(B):
        parts = [results[2 * b_idx + g]["out"] for g in range(2)]  # [6, 64, S]
        ctxT = np.concatenate(parts, axis=0)                       # [12, 64, S]
        final[b_idx] = ctxT.transpose(2, 0, 1).reshape(S, D)
    return final


def kernel(**inputs) -> np.ndarray:
    nc = build_program()
    in_maps = shard_inputs(**inputs)
    res = run_bass_kernel_spmd(nc, in_maps, core_ids=list(range(N_CORES)))
    return assemble_output(res.results)
